# revision 1
# baseline (speedup 1.0000x reference)
"""CrossAttention Trainium2 kernel (Bass/Tile), batch-parallel over 8 NeuronCores.

Problem (per batch b of 8):
    x   [512, 32, 32]  -> X   [C=512, N=1024]
    ctx [512, 32, 32]  -> CTX [C=512, M=1024]
    q = Wq@X * s + bq*s ; k = Wk@CTX + bk ; v = Wv@CTX + bv     (1x1 convs)
    per head h (8 heads x 64): simT[j,i] = sum_d k[d,j] q[d,i]
    attn = softmax_j(sim);  out[i,d] = sum_j attn[i,j] v[d,j]
    final = Wo@out + bo

Layout strategy (per core = one batch):
  - channels live on partitions in chunks of 128 (4 chunks); tokens on the free axis
  - sim is computed TRANSPOSED (j on partitions) so that the softmax denominator
    and the attn@v contraction both have j on partitions (PE contracts partitions)
  - v is computed transposed (vT[j, o]) directly by swapping matmul operands, and
    stored per head with a ones-column appended: attn@v with lhsT=[v_h | 1] gives
    both the (unnormalized) output AND the softmax denominator in one PSUM tile
  - exp runs on the scalar engine (ACT) draining 4-bank PSUM groups in one
    instruction to amortize the ~352-cycle per-instruction overhead
  - row-packed sim matmuls: head pairs use K=64 at base partitions 0/64 so both
    matmuls run concurrently in the PE array (row-group tiling)

Host-side prep (NOT device time): weights are pre-transposed and pre-cast to
bf16, the 1/sqrt(dim_head) scale is folded into Wq/bq.
"""

import contextlib
import os
import sys

sys.path.insert(0, "/opt/trn_rl_repo")

import numpy as np
import ml_dtypes

import concourse.bass as bass
import concourse.tile as tile
from concourse import bacc, mybir

B = 8
HEADS = 8
DH = 64
C = 512
NTOK = 1024  # 32*32
P = 128
CCH = C // P  # 4 channel chunks
JCH = NTOK // P  # 8 context-token chunks (partition dim of simT)
ICH = 2  # query-token chunks of 512 (free dim)
F = 512
SCALE = DH ** (-0.5)

BF16 = mybir.dt.bfloat16
F32 = mybir.dt.float32
NPBF16 = ml_dtypes.bfloat16


def build_nc(reps: int = 1):
    nc = bacc.Bacc("TRN2", target_bir_lowering=False, debug=False)

    x_d = nc.dram_tensor("x", [C, NTOK], BF16, kind="ExternalInput")
    c_d = nc.dram_tensor("ctx", [C, NTOK], BF16, kind="ExternalInput")
    wqt_d = nc.dram_tensor("wqt", [C, C], BF16, kind="ExternalInput")
    wkt_d = nc.dram_tensor("wkt", [C, C], BF16, kind="ExternalInput")
    wvt_d = nc.dram_tensor("wvt", [C, C], BF16, kind="ExternalInput")
    wot_d = nc.dram_tensor("wot", [C, C], BF16, kind="ExternalInput")
    bq_d = nc.dram_tensor("bq", [C], F32, kind="ExternalInput")
    bk_d = nc.dram_tensor("bk", [C], F32, kind="ExternalInput")
    bv_d = nc.dram_tensor("bv", [C], F32, kind="ExternalInput")
    bo_d = nc.dram_tensor("bo", [C], F32, kind="ExternalInput")
    out_d = nc.dram_tensor("out", [C, NTOK], F32, kind="ExternalOutput")

    with tile.TileContext(nc) as tc:
        with (
            tc.tile_pool(name="consts", bufs=1) as consts,
            tc.tile_pool(name="acts", bufs=1) as acts,
            tc.tile_pool(name="expp", bufs=3) as expp,
            tc.tile_pool(name="sbcp", bufs=6) as sbcp,
            tc.tile_pool(name="attsb", bufs=4) as attsb,
            tc.tile_pool(name="finp", bufs=2) as finp,
            tc.tile_pool(name="simA", bufs=1, space="PSUM") as simA,
            tc.tile_pool(name="simB", bufs=1, space="PSUM") as simB,
            tc.tile_pool(name="mxps", bufs=2, space="PSUM") as mxps,
        ):
          with (tc.For_i(0, reps, 1) if reps > 1 else contextlib.nullcontext()) as _i:
            # ---- constants ----
            wq_sb = consts.tile([P, CCH, C], BF16, tag="wq")
            wk_sb = consts.tile([P, CCH, C], BF16, tag="wk")
            wv_sb = consts.tile([P, CCH, C], BF16, tag="wv")
            wo_sb = consts.tile([P, CCH, C], BF16, tag="wo")

            bq_sb = consts.tile([P, CCH], F32, tag="bq")
            bk_sb = consts.tile([P, CCH], F32, tag="bk")
            bo_sb = consts.tile([P, CCH], F32, tag="bo")
            # bv broadcast across partitions: [128, 512] (free axis = channel)
            bv_bc = consts.tile([P, C], F32, tag="bvbc")
            b_ap = bv_d[None, :]
            bv_src = bass.AP(
                tensor=b_ap.tensor, offset=b_ap.offset, ap=[[0, P]] + list(b_ap.ap[1:])
            )
            nc.gpsimd.dma_start(out=bv_bc[:, :], in_=bv_src)

            # ---- activations (sync queue) + weights (scalar queue), interleaved
            # so the vT projection can start as soon as wv/ctx chunks land
            x_sb = acts.tile([P, CCH, NTOK], BF16, tag="x")
            c_sb = acts.tile([P, CCH, NTOK], BF16, tag="c")
            for cc in range(CCH):
                nc.sync.dma_start(out=c_sb[:, cc, :], in_=c_d[cc * P : (cc + 1) * P, :])
                nc.scalar.dma_start(out=wv_sb[:, cc, :], in_=wvt_d[cc * P : (cc + 1) * P, :])
            for b_sb, b_d in ((bq_sb, bq_d), (bk_sb, bk_d), (bo_sb, bo_d)):
                nc.sync.dma_start(out=b_sb[:, :], in_=b_d.rearrange("(a p) -> p a", p=P))
            for cc in range(CCH):
                nc.sync.dma_start(out=x_sb[:, cc, :], in_=x_d[cc * P : (cc + 1) * P, :])
                nc.scalar.dma_start(out=wq_sb[:, cc, :], in_=wqt_d[cc * P : (cc + 1) * P, :])
            for cc in range(CCH):
                nc.scalar.dma_start(out=wk_sb[:, cc, :], in_=wkt_d[cc * P : (cc + 1) * P, :])
                nc.scalar.dma_start(out=wo_sb[:, cc, :], in_=wot_d[cc * P : (cc + 1) * P, :])

            q_sb = acts.tile([P, CCH, NTOK], BF16, tag="q")
            k_sb = acts.tile([P, CCH, NTOK], BF16, tag="k")
            # vT with a ones column per head: [j-part, j-chunk, head, 64+1]
            vte_sb = acts.tile([P, JCH, HEADS, DH + 1], BF16, tag="vte")
            # attention output, [channel-part, pair-chunk, 512] per ic
            oall_sb = [
                acts.tile([P, CCH, F], BF16, tag=f"oall{ic}", name=f"oall{ic}")
                for ic in range(ICH)
            ]

            nc.vector.memset(vte_sb[:, :, :, DH : DH + 1], 1.0)

            # ---- vT projection (mm pool, single-bank groups) ----
            for mc in range(JCH):
                ps = mxps.tile([P, F], F32, tag="mx", name=f"vps{mc}")
                for cc in range(CCH):
                    nc.tensor.matmul(
                        ps[:, :],
                        c_sb[:, cc, mc * P : (mc + 1) * P],
                        wv_sb[:, cc, :],
                        start=(cc == 0),
                        stop=(cc == CCH - 1),
                    )
                nc.vector.tensor_tensor(
                    vte_sb[:, mc, :, 0:DH],
                    ps.rearrange("p (h d) -> p h d", d=DH),
                    bv_bc.rearrange("p (h d) -> p h d", d=DH),
                    mybir.AluOpType.add,
                )

            def emit_qk_group(which, oc, ih):
                dst, wt, bias_t, src_sb = (
                    (q_sb, wq_sb, bq_sb, x_sb) if which == "q" else (k_sb, wk_sb, bk_sb, c_sb)
                )
                ps = mxps.tile([P, F], F32, tag="mx", name=f"{which}ps{oc}{ih}")
                for cc in range(CCH):
                    nc.tensor.matmul(
                        ps[:, :],
                        wt[:, cc, oc * P : (oc + 1) * P],
                        src_sb[:, cc, ih * F : (ih + 1) * F],
                        start=(cc == 0),
                        stop=(cc == CCH - 1),
                    )
                nc.vector.tensor_tensor(
                    dst[:, oc, ih * F : (ih + 1) * F],
                    ps[:, :],
                    bias_t[:, oc : oc + 1].to_broadcast([P, F]),
                    mybir.AluOpType.add,
                )

            # q/k for the first head pair only; the rest stream inside the
            # attention loop (PE fills ACT-drain stalls with projection work)
            for ih in range(ICH):
                emit_qk_group("q", 0, ih)
            for ih in range(ICH):
                emit_qk_group("k", 0, ih)

            # ---- attention (software-pipelined, proj-merged) ----
            # Per (ic, pair) iteration the 16 sim tiles [128,512] go through
            # alternating 4-bank / 2-bank psum groups (A,B,A,B,A) so ACT gets
            # large exp instructions (2048/1024 els) while staying double-
            # buffered (A fills while B drains and vice versa). The attn@v
            # matmuls of the PREVIOUS pair are front-loaded into the first
            # steps so their psum slots (shared "mx" pool) free up for the
            # projection groups streamed later in the iteration.

            def emit_epilogue(pic, ppr, pes, pats):
                for hb in range(2):
                    at_sb = attsb.tile([DH + 1, F], F32, tag="atsb", name=f"atsb{pic}{ppr}{hb}")
                    nc.vector.tensor_copy(at_sb[:, :], pats[hb][0 : DH + 1, :])
                    den1 = sbcp.tile([1, F], F32, tag="den1", name=f"den1{pic}{ppr}{hb}")
                    nc.vector.reciprocal(out=den1[:, :], in_=at_sb[DH : DH + 1, :])
                    sden = sbcp.tile([DH, F], F32, tag="sden", name=f"sden{pic}{ppr}{hb}")
                    nc.gpsimd.partition_broadcast(sden[:, :], den1[:, :])
                    nc.vector.tensor_tensor(
                        oall_sb[pic][hb * DH : (hb + 1) * DH, ppr, :],
                        at_sb[0:DH, :],
                        sden[:, :],
                        mybir.AluOpType.mult,
                    )

            def emit_oproj(ic, ocs):
                for oc in ocs:
                    ps = mxps.tile([P, F], F32, tag="mx", name=f"ops{ic}{oc}")
                    for cc in range(CCH):
                        nc.tensor.matmul(
                            ps[:, :],
                            wo_sb[:, cc, oc * P : (oc + 1) * P],
                            oall_sb[ic][:, cc, :],
                            start=(cc == 0),
                            stop=(cc == CCH - 1),
                        )
                    fin = finp.tile([P, F], F32, tag="fin", name=f"fin{ic}{oc}")
                    nc.vector.tensor_tensor(
                        fin[:, :],
                        ps[:, :],
                        bo_sb[:, oc : oc + 1].to_broadcast([P, F]),
                        mybir.AluOpType.add,
                    )
                    nc.sync.dma_start(
                        out=out_d[oc * P : (oc + 1) * P, ic * F : (ic + 1) * F],
                        in_=fin[:, :],
                    )

            STEPS = (("A", (0, 1)), ("B", (2,)), ("A", (3, 4)), ("B", (5,)), ("A", (6, 7)))

            def emit_iteration(ic, pr, es, prev, pats, proj_jobs, self_pats=None):
                # attn queue for the previous pair: front-loaded 4 per step
                attn_q = []
                if prev is not None:
                    pic, ppr, pes = prev
                    for jc in range(JCH):
                        for hb in range(2):
                            attn_q.append((hb, jc))
                for si, (kind, jcs) in enumerate(STEPS):
                    for _ in range(4):
                        if attn_q:
                            hb, jc = attn_q.pop(0)
                            nc.tensor.matmul(
                                pats[hb][0 : DH + 1, :],
                                vte_sb[:, jc, 2 * ppr + hb, :],
                                pes[:, jc, hb, :],
                                start=(jc == 0),
                                stop=(jc == JCH - 1),
                            )
                    pool = simA if kind == "A" else simB
                    nb = 2 * len(jcs)
                    g = pool.tile([P, nb, F], F32, tag=kind, name=f"g{ic}{pr}{si}")
                    for idx, jc in enumerate(jcs):
                        for hb in range(2):
                            nc.tensor.matmul(
                                g[:, 2 * idx + hb, :],
                                k_sb[hb * DH : (hb + 1) * DH, pr, jc * P : (jc + 1) * P],
                                q_sb[hb * DH : (hb + 1) * DH, pr, ic * F : (ic + 1) * F],
                                start=True,
                                stop=True,
                            )
                    nc.scalar.activation(
                        out=es[:, jcs[0] : jcs[-1] + 1, :, :],
                        in_=g[:, :, :],
                        func=mybir.ActivationFunctionType.Exp,
                    )
                    if si == 3:
                        if prev is not None:
                            emit_epilogue(pic, ppr, pes, pats)
                            for job in proj_jobs:
                                job()
                if self_pats is not None:
                    # chase this iteration's own attn for already-exp'd jc
                    for jc in range(6):
                        for hb in range(2):
                            nc.tensor.matmul(
                                self_pats[hb][0 : DH + 1, :],
                                vte_sb[:, jc, 2 * pr + hb, :],
                                es[:, jc, hb, :],
                                start=(jc == 0),
                                stop=False,
                            )
                if prev is None:
                    for job in proj_jobs:
                        job()

            iters = [(ic, pr) for ic in range(ICH) for pr in range(CCH)]
            prev = None
            for ic, pr in iters:
                es = expp.tile([P, JCH, 2, F], BF16, tag="es", name=f"es{ic}{pr}")
                pats = None
                if prev is not None:
                    pic0, ppr0, _ = prev
                    pats = [
                        mxps.tile([P, F], F32, tag="mx", name=f"at{pic0}{ppr0}{hb}")
                        for hb in range(2)
                    ]
                proj_jobs = []
                if ic == 0 and pr < CCH - 1:
                    for which in ("q", "k"):
                        for ih in range(ICH):
                            proj_jobs.append(
                                lambda w=which, o=pr + 1, i=ih: emit_qk_group(w, o, i)
                            )
                elif (ic, pr) == (1, 0):
                    proj_jobs.append(lambda: emit_oproj(0, (0, 1)))
                elif (ic, pr) == (1, 1):
                    proj_jobs.append(lambda: emit_oproj(0, (2, 3)))
                self_pats = None
                if (ic, pr) == iters[-1]:
                    self_pats = [
                        mxps.tile([P, F], F32, tag="mx", name=f"atL{hb}")
                        for hb in range(2)
                    ]
                    last_pats = self_pats
                emit_iteration(ic, pr, es, prev, pats, proj_jobs, self_pats)
                prev = (ic, pr, es)

            # finish the last pair: remaining jc of the self-chase
            pic, ppr, pes = prev
            for jc in range(6, JCH):
                for hb in range(2):
                    nc.tensor.matmul(
                        last_pats[hb][0 : DH + 1, :],
                        vte_sb[:, jc, 2 * ppr + hb, :],
                        pes[:, jc, hb, :],
                        start=False,
                        stop=(jc == JCH - 1),
                    )
            emit_epilogue(pic, ppr, pes, last_pats)
            emit_oproj(1, (0, 1, 2, 3))

    nc.compile()
    return nc


def prep_inputs(x, context, Wq, bq, Wk, bk, Wv, bv, Wo, bo):
    """Host-side sharding + layout prep. Returns per-core input maps."""
    xb = np.asarray(x, np.float32).reshape(B, C, NTOK).astype(NPBF16)
    cb = np.asarray(context, np.float32).reshape(B, C, NTOK).astype(NPBF16)
    wqt = np.ascontiguousarray((np.asarray(Wq, np.float32) * SCALE).T).astype(NPBF16)
    wkt = np.ascontiguousarray(np.asarray(Wk, np.float32).T).astype(NPBF16)
    wvt = np.ascontiguousarray(np.asarray(Wv, np.float32).T).astype(NPBF16)
    wot = np.ascontiguousarray(np.asarray(Wo, np.float32).T).astype(NPBF16)
    bqs = (np.asarray(bq, np.float32) * SCALE).astype(np.float32)
    bkf = np.asarray(bk, np.float32)
    bvf = np.asarray(bv, np.float32)
    bof = np.asarray(bo, np.float32)
    in_maps = []
    for b in range(B):
        in_maps.append(
            {
                "x": np.ascontiguousarray(xb[b]),
                "ctx": np.ascontiguousarray(cb[b]),
                "wqt": wqt,
                "wkt": wkt,
                "wvt": wvt,
                "wot": wot,
                "bq": bqs,
                "bk": bkf,
                "bv": bvf,
                "bo": bof,
            }
        )
    return in_maps


_NC = None


def _get_nc():
    global _NC
    if _NC is None:
        _NC = build_nc()
    return _NC


def kernel(x, context, Wq, bq, Wk, bk, Wv, bv, Wo, bo):
    from concourse.bass_utils import run_bass_kernel_spmd

    nc = _get_nc()
    in_maps = prep_inputs(x, context, Wq, bq, Wk, bk, Wv, bv, Wo, bo)
    br = run_bass_kernel_spmd(nc, in_maps, list(range(B)))
    out = np.stack([np.asarray(br.results[b]["out"], np.float32) for b in range(B)])
    return out.reshape(B, C, 32, 32)



# revision 23
# speedup vs baseline: 1.1585x; 1.1585x over previous
"""CrossAttention Trainium2 kernel (Bass/Tile), batch-parallel over 8 NeuronCores.

Problem (per batch b of 8):
    x   [512, 32, 32]  -> X   [C=512, N=1024]
    ctx [512, 32, 32]  -> CTX [C=512, M=1024]
    q = Wq@X + bq ; k = Wk@CTX + bk ; v = Wv@CTX + bv     (1x1 convs)
    per head h (8 heads x 64): simT[j,i] = s * sum_d k[d,j] q[d,i]
    attn = softmax_j(sim);  out[i,d] = sum_j attn[i,j] v[d,j]
    final = Wo@out + bo

v2 strategy (per core = one batch):
  - projections bf16 (precision-critical), channels on partitions
  - q/k cast to fp8e4 staging [128, pair, tok]; pairs 2-3 DMA-folded into a
    zero-padded DoubleRow layout [64, pr, hb, ko2, tok] (sim cost 256 cyc);
    pairs 0-1 run plain fp8 sim from staging (512 cyc) to avoid fold races
  - sim transposed (j on partitions); exp on ACT applies the 1/8 scale free;
    es bf16 [128, jc, hb, 512]
  - attn@v with i on PSUM partitions: lhsT = es slice [j,128i], rhs =
    vte slice [j,64] -> cost 64/matmul; denominators via rhs=ones [j,1]
    (cost 1/matmul) accumulated in the same PSUM bank
  - normalize: one reciprocal + one batched 4D multiply per att tile
  - v projected in normal orientation, then DMA-XBAR-transposed into
    vte [j, pair, jc, c]; attention output DMA-XBAR-transposed into oall
  - PSUM pools: sim 2x2 banks, att 2x1, proj 2x1 = 8
  - dummy warmup matmuls at t=0 hold the PE p-state ramp at full speed
"""

import contextlib
import os
import sys

sys.path.insert(0, "/opt/trn_rl_repo")

import numpy as np
import ml_dtypes

import concourse.bass as bass
import concourse.tile as tile
from concourse import bacc, mybir

B = 8
HEADS = 8
DH = 64
C = 512
NTOK = 1024
P = 128
CCH = C // P  # 4 head-pair chunks
JCH = NTOK // P  # 8 context-token chunks
ICH = 2
F = 512
SCALE = DH ** (-0.5)

BF16 = mybir.dt.bfloat16
F32 = mybir.dt.float32
FP8 = mybir.dt.float8e4
NPBF16 = ml_dtypes.bfloat16
DR = mybir.MatmulPerfMode.DoubleRow


def rawap(base, dims, extra_offset=0):
    return bass.AP(tensor=base.tensor, offset=base.offset + extra_offset, ap=dims)


def build_nc(reps: int = 1):
    nc = bacc.Bacc("TRN2", target_bir_lowering=False, debug=False)

    x_d = nc.dram_tensor("x", [C, NTOK], BF16, kind="ExternalInput")
    c_d = nc.dram_tensor("ctx", [C, NTOK], BF16, kind="ExternalInput")
    wqt_d = nc.dram_tensor("wqt", [C, C], BF16, kind="ExternalInput")
    wkt_d = nc.dram_tensor("wkt", [C, C], BF16, kind="ExternalInput")
    wvt_d = nc.dram_tensor("wvt", [C, C], BF16, kind="ExternalInput")
    wot_d = nc.dram_tensor("wot", [C, C], BF16, kind="ExternalInput")
    bq_d = nc.dram_tensor("bq", [C], F32, kind="ExternalInput")
    bk_d = nc.dram_tensor("bk", [C], F32, kind="ExternalInput")
    bv_d = nc.dram_tensor("bv", [C], F32, kind="ExternalInput")
    bo_d = nc.dram_tensor("bo", [C], F32, kind="ExternalInput")
    out_d = nc.dram_tensor("out", [C, NTOK], F32, kind="ExternalOutput")

    with tile.TileContext(nc) as tc:
        with (
            tc.tile_pool(name="consts", bufs=1) as consts,
            tc.tile_pool(name="acts", bufs=1) as acts,
            tc.tile_pool(name="expp", bufs=2) as expp,
            tc.tile_pool(name="sbcp", bufs=4) as sbcp,
            tc.tile_pool(name="onp", bufs=2) as onp,
            tc.tile_pool(name="finp", bufs=2) as finp,
            tc.tile_pool(name="simP", bufs=2, space="PSUM") as simP,
            tc.tile_pool(name="attP", bufs=2, space="PSUM") as attP,
            tc.tile_pool(name="projP", bufs=2, space="PSUM") as projP,
        ):
          with (tc.For_i(0, reps, 1) if reps > 1 else contextlib.nullcontext()) as _i:
            # ---- PE p-state warmup (no DMA dependency) ----
            warm_sb = consts.tile([P, DH], BF16, tag="warm")
            nc.vector.memset(warm_sb[:, :], 0.0)
            wps = projP.tile([P, F], F32, tag="pj", name="warmps")
            for i in range(20):
                nc.tensor.matmul(
                    wps[0:DH, 0:DH], warm_sb[:, :], warm_sb[:, :], start=True, stop=True
                )

            # ---- constants / inputs ----
            wq_sb = consts.tile([P, CCH, C], BF16, tag="wq")
            wk_sb = consts.tile([P, CCH, C], BF16, tag="wk")
            wv_sb = consts.tile([P, CCH, C], BF16, tag="wv")
            wo_sb = consts.tile([P, CCH, C], BF16, tag="wo")
            bq_sb = consts.tile([P, CCH], F32, tag="bq")
            bk_sb = consts.tile([P, CCH], F32, tag="bk")
            bv_sb = consts.tile([P, CCH], F32, tag="bv")
            bo_sb = consts.tile([P, CCH], F32, tag="bo")
            ones_sb = consts.tile([P, 1], BF16, tag="ones")
            nc.vector.memset(ones_sb[:, :], 1.0)

            x_sb = acts.tile([P, CCH, NTOK], BF16, tag="x")
            c_sb = acts.tile([P, CCH, NTOK], BF16, tag="c")
            # fine-grained, dependency-ordered input DMAs: everything the
            # first k-projection group needs lands first
            wkt_r = wkt_d.rearrange("(cc p) n -> p cc n", p=P)
            wqt_r = wqt_d.rearrange("(cc p) n -> p cc n", p=P)
            wvt_r = wvt_d.rearrange("(cc p) n -> p cc n", p=P)
            wot_r = wot_d.rearrange("(cc p) n -> p cc n", p=P)
            c_r = c_d.rearrange("(cc p) n -> p cc n", p=P)
            x_r = x_d.rearrange("(cc p) n -> p cc n", p=P)
            # critical path first, in token-halves: the first k/q projection
            # groups (pair 0, tokens 0:512) unblock the first sim tiles
            nc.sync.dma_start(out=bk_sb[:, :], in_=bk_d.rearrange("(a p) -> p a", p=P))
            nc.scalar.dma_start(out=wk_sb[:, :, 0:P], in_=wkt_r[:, :, 0:P])
            nc.sync.dma_start(out=c_sb[:, :, 0:F], in_=c_r[:, :, 0:F])
            nc.scalar.dma_start(out=bq_sb[:, :], in_=bq_d.rearrange("(a p) -> p a", p=P))
            nc.scalar.dma_start(out=wq_sb[:, :, 0:P], in_=wqt_r[:, :, 0:P])
            nc.sync.dma_start(out=x_sb[:, :, 0:F], in_=x_r[:, :, 0:F])
            nc.sync.dma_start(out=c_sb[:, :, F:NTOK], in_=c_r[:, :, F:NTOK])
            nc.sync.dma_start(out=x_sb[:, :, F:NTOK], in_=x_r[:, :, F:NTOK])
            for b_sb, b_dd in ((bv_sb, bv_d), (bo_sb, bo_d)):
                nc.scalar.dma_start(out=b_sb[:, :], in_=b_dd.rearrange("(a p) -> p a", p=P))
            # remaining weight columns (pairs 1-3) and wv/wo
            nc.scalar.dma_start(out=wk_sb[:, :, P:C], in_=wkt_r[:, :, P:C])
            nc.scalar.dma_start(out=wq_sb[:, :, P:C], in_=wqt_r[:, :, P:C])
            nc.scalar.dma_start(out=wv_sb[:, :, :], in_=wvt_r[:, :, :])
            nc.scalar.dma_start(out=wo_sb[:, :, :], in_=wot_r[:, :, :])

            # bf16 q/k staging [c-part, pair, tok]
            qst = acts.tile([P, CCH, NTOK], BF16, tag="qst")
            kst = acts.tile([P, CCH, NTOK], BF16, tag="kst")

            # v: bf16 staging [c, pair, j] then XBAR-transposed vte [j, pair, jc, c]
            vst = acts.tile([P, CCH, NTOK], BF16, tag="vst")
            vte = acts.tile([P, CCH, JCH, P], BF16, tag="vte")

            oall_sb = [
                acts.tile([P, CCH, F], BF16, tag=f"oall{ic}", name=f"oall{ic}")
                for ic in range(ICH)
            ]

            # ---- job emitters ----
            def emit_proj_group(which, pr, ih):
                """bf16 projection group -> staging (q8st/k8st fp8, vst bf16)."""
                dst, wt, bias_t, src_sb = {
                    "q": (qst, wq_sb, bq_sb, x_sb),
                    "k": (kst, wk_sb, bk_sb, c_sb),
                    "v": (vst, wv_sb, bv_sb, c_sb),
                }[which]
                ps = projP.tile([P, F], F32, tag="pj", name=f"{which}ps{pr}{ih}")
                for cc in range(CCH):
                    nc.tensor.matmul(
                        ps[:, :],
                        wt[:, cc, pr * P : (pr + 1) * P],
                        src_sb[:, cc, ih * F : (ih + 1) * F],
                        start=(cc == 0),
                        stop=(cc == CCH - 1),
                    )
                nc.vector.tensor_tensor(
                    dst[:, pr, ih * F : (ih + 1) * F],
                    ps[:, :],
                    bias_t[:, pr : pr + 1].to_broadcast([P, F]),
                    mybir.AluOpType.add,
                )

            def emit_vtr(pr, jh):
                nc.sync.dma_start_transpose(
                    vte[:, pr, jh * 4 : (jh + 1) * 4, :],
                    vst[:, pr, jh * F : (jh + 1) * F],
                )

            # split final (ic=1) o-projection: pairs 0-2 accumulate into SBUF
            # partials during the last iteration; only pair 3 remains at the tail
            partials = acts.tile([P, CCH, F], F32, tag="partials")
            opart_ps = {}

            def emit_opart(oc, ccs, finish):
                if oc not in opart_ps:
                    opart_ps[oc] = projP.tile([P, F], F32, tag="pj", name=f"opp{oc}")
                ps = opart_ps[oc]
                for cc in ccs:
                    nc.tensor.matmul(
                        ps[:, :],
                        wo_sb[:, cc, oc * P : (oc + 1) * P],
                        oall_sb[1][:, cc, :],
                        start=(cc == 0),
                        stop=(finish and cc == ccs[-1]),
                    )
                if finish:
                    nc.vector.tensor_tensor(
                        partials[:, oc, :],
                        ps[:, :],
                        bo_sb[:, oc : oc + 1].to_broadcast([P, F]),
                        mybir.AluOpType.add,
                    )

            fin_tail = acts.tile([P, CCH, F], F32, tag="fintail")

            def emit_oproj_tail(oc):
                ps = projP.tile([P, F], F32, tag="pj", name=f"opt{oc}")
                nc.tensor.matmul(
                    ps[:, :],
                    wo_sb[:, 3, oc * P : (oc + 1) * P],
                    oall_sb[1][:, 3, :],
                    start=True,
                    stop=True,
                )
                nc.vector.tensor_tensor(
                    fin_tail[:, oc, :], ps[:, :], partials[:, oc, :], mybir.AluOpType.add
                )
                if oc in (1, 3):
                    nc.sync.dma_start(
                        out=out_d.rearrange("(cc p) n -> p cc n", p=P)[
                            :, oc - 1 : oc + 1, F : 2 * F
                        ],
                        in_=fin_tail[:, oc - 1 : oc + 1, :],
                    )

            def emit_oproj(ic, oc):
                ps = projP.tile([P, F], F32, tag="pj", name=f"ops{ic}{oc}")
                for cc in range(CCH):
                    nc.tensor.matmul(
                        ps[:, :],
                        wo_sb[:, cc, oc * P : (oc + 1) * P],
                        oall_sb[ic][:, cc, :],
                        start=(cc == 0),
                        stop=(cc == CCH - 1),
                    )
                fin = finp.tile([P, F], F32, tag="fin", name=f"fin{ic}{oc}")
                nc.vector.tensor_tensor(
                    fin[:, :],
                    ps[:, :],
                    bo_sb[:, oc : oc + 1].to_broadcast([P, F]),
                    mybir.AluOpType.add,
                )
                nc.sync.dma_start(
                    out=out_d[oc * P : (oc + 1) * P, ic * F : (ic + 1) * F],
                    in_=fin[:, :],
                )

            def emit_sim(g, slot, pr, hb, jc, ic):
                nc.tensor.matmul(
                    g[:, slot, :],
                    kst[hb * DH : (hb + 1) * DH, pr, jc * P : (jc + 1) * P],
                    qst[hb * DH : (hb + 1) * DH, pr, ic * F : (ic + 1) * F],
                    start=True,
                    stop=True,
                )

            # ---- attn@v chain machinery (for pair `prev`, runs one iter later)
            def make_state(prev):
                pic, ppr, pes = prev
                t0 = attP.tile([P, F], F32, tag="at", name=f"at{pic}{ppr}0")
                t1 = attP.tile([P, F], F32, tag="at", name=f"at{pic}{ppr}1")
                onT = onp.tile([P, CCH, P], BF16, tag="onT", name=f"onT{pic}{ppr}")
                return {
                    "q": [(slot, jc) for jc in range(JCH) for slot in range(8)],
                    "tiles": [t0, t1],
                    "prev": prev,
                    "onT": onT,
                    # one accumulation group per bank: count matmuls per tile
                    "nmm": [0, 0],
                }

            def emit_chains_block(state, n):
                pic, ppr, pes = state["prev"]
                for _ in range(n):
                    if not state["q"]:
                        return
                    slot, jc = state["q"].pop(0)
                    half = slot // 4
                    tl = state["tiles"][half]
                    ichunk, hb = slot >> 1, slot & 1
                    col = (slot % 4 >> 1) * P + hb * DH
                    lhsT = pes[:, jc, hb, ichunk * P : (ichunk + 1) * P]
                    n = state["nmm"][half]
                    nc.tensor.matmul(
                        tl[:, col : col + DH],
                        lhsT,
                        vte[:, ppr, jc, hb * DH : (hb + 1) * DH],
                        start=(n == 0),
                        stop=False,
                        skip_group_check=True,
                    )
                    nc.tensor.matmul(
                        tl[:, 256 + (slot % 4) : 257 + (slot % 4)],
                        lhsT,
                        ones_sb[:, :],
                        start=False,
                        stop=(n + 1 == 63),
                        skip_group_check=True,
                    )
                    state["nmm"][half] = n + 2

            def emit_norm(state, half):
                pic, ppr, _ = state["prev"]
                tl = state["tiles"][half]
                onT = state["onT"]
                base = tl[:, :]
                rc = sbcp.tile([P, 4], F32, tag="rc", name=f"rc{pic}{ppr}{half}")
                nc.vector.reciprocal(
                    out=rc[:, :],
                    in_=rawap(base, [list(base.ap[0]), [1, 4]], extra_offset=256),
                )
                on0 = onT[:, half * 2, 0:DH]
                ch_ap = rawap(base, [list(base.ap[0]), [P, 2], [DH, 2], [1, DH]])
                on_ap = rawap(on0, [list(on0.ap[0]), [P, 2], [DH, 2], [1, DH]])
                rcb = rc[:, :]
                rc_ap = rawap(rcb, [list(rcb.ap[0]), [2, 2], [1, 2], [0, DH]])
                nc.vector.tensor_tensor(on_ap, ch_ap, rc_ap, mybir.AluOpType.mult)

            def emit_otr(state):
                pic, ppr, _ = state["prev"]
                nc.sync.dma_start_transpose(
                    oall_sb[pic].rearrange("p cc (a b) -> p cc a b", a=CCH)[:, ppr, :, :],
                    state["onT"].rearrange("p a b -> p (a b)"),
                )

            # ---- prologue: kq0 + kq1 around the first sim steps ----
            emit_proj_group("k", 0, 0)
            emit_proj_group("q", 0, 0)
            emit_proj_group("k", 0, 1)

            # global job queue with soft per-iteration targets
            jobs = []
            jobs += [lambda i=i: emit_proj_group("k", 1, i) for i in range(ICH)]
            jobs += [lambda: emit_proj_group("q", 1, 0)]
            jobs += [lambda i=i: emit_proj_group("v", 0, i) for i in range(ICH)]
            jobs += [lambda j=j: emit_vtr(0, j) for j in range(2)]
            jobs += [lambda i=i: emit_proj_group("k", 2, i) for i in range(ICH)]
            jobs += [lambda: emit_proj_group("q", 2, 0)]
            jobs += [lambda i=i: emit_proj_group("v", 1, i) for i in range(ICH)]
            jobs += [lambda j=j: emit_vtr(1, j) for j in range(2)]
            jobs += [lambda i=i: emit_proj_group("k", 3, i) for i in range(ICH)]
            jobs += [lambda: emit_proj_group("q", 3, 0)]
            jobs += [lambda i=i: emit_proj_group("v", 2, i) for i in range(ICH)]
            jobs += [lambda j=j: emit_vtr(2, j) for j in range(2)]
            jobs += [lambda: emit_proj_group("q", 0, 1)]
            jobs += [lambda: emit_proj_group("q", 1, 1)]
            jobs += [lambda i=i: emit_proj_group("v", 3, i) for i in range(ICH)]
            jobs += [lambda j=j: emit_vtr(3, j) for j in range(2)]
            jobs += [lambda: emit_proj_group("q", 2, 1)]
            jobs += [lambda: emit_proj_group("q", 3, 1)]
            jobs += [lambda o=o: emit_oproj(0, o) for o in range(4)]
            # cumulative job targets by END of iteration 0..7
            targets = [7, 14, 21, 28, 28, 31, 33, 33]

            jobs_done = 0
            iters = [(ic, pr) for ic in range(ICH) for pr in range(CCH)]
            prev = None
            state = None
            for it_idx, (ic, pr) in enumerate(iters):
                es = expp.tile([P, JCH, 2, F], BF16, tag="es", name=f"es{ic}{pr}")
                state = make_state(prev) if prev is not None else None
                tgt = targets[it_idx]
                start_jobs = jobs_done
                for t in range(JCH):  # 8 sim tiles per iteration
                    g = simP.tile([P, 2, F], F32, tag="sim", name=f"g{ic}{pr}{t}")
                    for hb in range(2):
                        emit_sim(g, hb, pr, hb, t, ic)
                    nc.scalar.activation(
                        out=es[:, t, :, :],
                        in_=g[:, :, :],
                        func=mybir.ActivationFunctionType.Exp,
                        scale=SCALE,
                    )
                    # spread this iteration's job quota evenly across the 8 steps
                    want = start_jobs + (tgt - start_jobs) * (t + 1) // JCH
                    while jobs_done < want and jobs_done < len(jobs):
                        jobs[jobs_done]()
                        jobs_done += 1
                    if it_idx == len(iters) - 1:
                        # last iteration: drain prev fast, then self-chase
                        # this pair's chains one exp-step behind
                        if t <= 3:
                            emit_chains_block(state, 16)
                        elif t == 4:
                            emit_chains_block(state, 99)
                            emit_norm(state, 0)
                            emit_norm(state, 1)
                            emit_otr(state)
                            self_state = make_state((ic, pr, es))
                            emit_chains_block(self_state, 32)  # backlog jc 0..3
                        else:
                            emit_chains_block(self_state, 8)  # jc = t-1
                            if t == 5:
                                emit_opart(0, [0, 1], False)
                            elif t == 6:
                                emit_opart(1, [0, 1], False)
                            elif t == 7:
                                emit_opart(0, [2], True)
                                emit_opart(1, [2], True)
                    elif state is not None and t >= 2:
                        emit_chains_block(state, 16)
                        if t == 5:
                            emit_norm(state, 0)
                            emit_norm(state, 1)
                            emit_otr(state)
                prev = (ic, pr, es)

            # ---- tail: last jc of the self-chase + final o-projection ----
            emit_chains_block(self_state, 99)
            emit_norm(self_state, 0)
            # half-0 transpose can start while half-1 normalizes
            pic, ppr, _ = self_state["prev"]
            nc.sync.dma_start_transpose(
                oall_sb[pic].rearrange("p cc (a b) -> p cc a b", a=CCH)[:, ppr, 0:2, :],
                self_state["onT"][:, 0:2, :].rearrange("p a b -> p (a b)"),
            )
            emit_norm(self_state, 1)
            nc.sync.dma_start_transpose(
                oall_sb[pic].rearrange("p cc (a b) -> p cc a b", a=CCH)[:, ppr, 2:4, :],
                self_state["onT"][:, 2:4, :].rearrange("p a b -> p (a b)"),
            )
            emit_opart(2, [0, 1, 2], True)
            emit_opart(3, [0, 1, 2], True)
            for oc in range(4):
                emit_oproj_tail(oc)

    nc.compile()
    return nc


def prep_inputs(x, context, Wq, bq, Wk, bk, Wv, bv, Wo, bo):
    xb = np.asarray(x, np.float32).reshape(B, C, NTOK).astype(NPBF16)
    cb = np.asarray(context, np.float32).reshape(B, C, NTOK).astype(NPBF16)
    wqt = np.ascontiguousarray(np.asarray(Wq, np.float32).T).astype(NPBF16)
    wkt = np.ascontiguousarray(np.asarray(Wk, np.float32).T).astype(NPBF16)
    wvt = np.ascontiguousarray(np.asarray(Wv, np.float32).T).astype(NPBF16)
    wot = np.ascontiguousarray(np.asarray(Wo, np.float32).T).astype(NPBF16)
    in_maps = []
    for b in range(B):
        in_maps.append(
            {
                "x": np.ascontiguousarray(xb[b]),
                "ctx": np.ascontiguousarray(cb[b]),
                "wqt": wqt,
                "wkt": wkt,
                "wvt": wvt,
                "wot": wot,
                "bq": np.asarray(bq, np.float32),
                "bk": np.asarray(bk, np.float32),
                "bv": np.asarray(bv, np.float32),
                "bo": np.asarray(bo, np.float32),
            }
        )
    return in_maps


_NC = None


def _get_nc():
    global _NC
    if _NC is None:
        _NC = build_nc()
    return _NC


def kernel(x, context, Wq, bq, Wk, bk, Wv, bv, Wo, bo):
    from concourse.bass_utils import run_bass_kernel_spmd

    nc = _get_nc()
    in_maps = prep_inputs(x, context, Wq, bq, Wk, bk, Wv, bv, Wo, bo)
    br = run_bass_kernel_spmd(nc, in_maps, list(range(B)))
    out = np.stack([np.asarray(br.results[b]["out"], np.float32) for b in range(B)])
    return out.reshape(B, C, 32, 32)


# revision 33
# speedup vs baseline: 1.2071x; 1.0419x over previous
"""CrossAttention Trainium2 kernel (Bass/Tile), batch-parallel over 8 NeuronCores.

Problem (per batch b of 8):
    x   [512, 32, 32]  -> X   [C=512, N=1024]
    ctx [512, 32, 32]  -> CTX [C=512, M=1024]
    q = Wq@X + bq ; k = Wk@CTX + bk ; v = Wv@CTX + bv     (1x1 convs)
    per head h (8 heads x 64): simT[j,i] = s * sum_d k[d,j] q[d,i]
    attn = softmax_j(sim);  out[i,d] = sum_j attn[i,j] v[d,j]
    final = Wo@out + bo

Layout strategy (per core = one batch), tuned for the timeline cost model
(matmul cost ~ output free size; LDWEIGHTS free):
  - all matmuls bf16 (fp8 q/k measured 5x the error budget); channels on
    partitions for projections, staged [c, pair, tok]
  - sim computed TRANSPOSED (j on partitions); exp on ACT applies the
    1/sqrt(dh) scale for free; es bf16 [j, jc, hb, i]
  - attn@v with i on PSUM partitions: lhsT = es slice [j, 128i], rhs =
    vte slice [j, 64] -> 64-cycle matmuls (vs 512 for the d-on-partitions
    orientation); denominators via rhs = ones [j,1] at 1 cycle/matmul,
    packed into the same PSUM bank as one per-element accumulation group
  - normalize: one reciprocal + one batched 4D multiply per 4 chains
    (denominator is a per-partition scalar in this orientation)
  - v projected in normal orientation then XBAR-DMA-transposed into
    vte [j, pair, jc, c]; attention output likewise XBAR-transposed into
    oall [c, pair, i]; the last pair uses a PE transpose (identity matmul)
    to skip the DMA latency on the critical tail
  - final o-projection split: pairs 0-2 accumulate into SBUF partials
    during the last iteration, only pair 3 + bf16 store remain at the tail
  - software pipeline: iteration (ic, pair) computes sim+exp while the
    previous pair's attn@v chains, norms, and transpose run; projection
    jobs stream through a deadline-ordered queue; dummy warmup matmuls
    hold the PE p-state ramp at full clock through idle windows
  - PSUM: sim 2x2-bank tiles (exp double-buffer), attn 2x1, proj 2x1
"""

import contextlib
import os
import sys

sys.path.insert(0, "/opt/trn_rl_repo")

import numpy as np
import ml_dtypes

import concourse.bass as bass
import concourse.tile as tile
from concourse import bacc, mybir

B = 8
HEADS = 8
DH = 64
C = 512
NTOK = 1024
P = 128
CCH = C // P  # 4 head-pair chunks
JCH = NTOK // P  # 8 context-token chunks
ICH = 2
F = 512
SCALE = DH ** (-0.5)

BF16 = mybir.dt.bfloat16
F32 = mybir.dt.float32
FP8 = mybir.dt.float8e4
NPBF16 = ml_dtypes.bfloat16
DR = mybir.MatmulPerfMode.DoubleRow


def rawap(base, dims, extra_offset=0):
    return bass.AP(tensor=base.tensor, offset=base.offset + extra_offset, ap=dims)


def build_nc(reps: int = 1):
    nc = bacc.Bacc("TRN2", target_bir_lowering=False, debug=False)

    x_d = nc.dram_tensor("x", [C, NTOK], BF16, kind="ExternalInput")
    c_d = nc.dram_tensor("ctx", [C, NTOK], BF16, kind="ExternalInput")
    wqt_d = nc.dram_tensor("wqt", [C, C], BF16, kind="ExternalInput")
    wkt_d = nc.dram_tensor("wkt", [C, C], BF16, kind="ExternalInput")
    wvt_d = nc.dram_tensor("wvt", [C, C], BF16, kind="ExternalInput")
    wot_d = nc.dram_tensor("wot", [C, C], BF16, kind="ExternalInput")
    bq_d = nc.dram_tensor("bq", [C], F32, kind="ExternalInput")
    bk_d = nc.dram_tensor("bk", [C], F32, kind="ExternalInput")
    bv_d = nc.dram_tensor("bv", [C], F32, kind="ExternalInput")
    bo_d = nc.dram_tensor("bo", [C], F32, kind="ExternalInput")
    id_d = nc.dram_tensor("ident", [P, P], BF16, kind="ExternalInput")
    out_d = nc.dram_tensor("out", [C, NTOK], BF16, kind="ExternalOutput")

    with tile.TileContext(nc) as tc:
        with (
            tc.tile_pool(name="consts", bufs=1) as consts,
            tc.tile_pool(name="acts", bufs=1) as acts,
            tc.tile_pool(name="expp", bufs=2) as expp,
            tc.tile_pool(name="sbcp", bufs=4) as sbcp,
            tc.tile_pool(name="onp", bufs=2) as onp,
            tc.tile_pool(name="finp", bufs=2) as finp,
            tc.tile_pool(name="simP", bufs=2, space="PSUM") as simP,
            tc.tile_pool(name="attP", bufs=2, space="PSUM") as attP,
            tc.tile_pool(name="projP", bufs=2, space="PSUM") as projP,
        ):
          with (tc.For_i(0, reps, 1) if reps > 1 else contextlib.nullcontext()) as _i:
            # ---- PE p-state warmup (no DMA dependency) ----
            warm_sb = consts.tile([P, DH], BF16, tag="warm")
            nc.vector.memset(warm_sb[:, :], 0.0)
            wps = projP.tile([P, F], F32, tag="pj", name="warmps")
            for i in range(60):
                nc.tensor.matmul(
                    wps[0:DH, 0:DH], warm_sb[:, :], warm_sb[:, :], start=True, stop=True
                )

            # ---- constants / inputs ----
            wq_sb = consts.tile([P, CCH, C], BF16, tag="wq")
            wk_sb = consts.tile([P, CCH, C], BF16, tag="wk")
            wv_sb = consts.tile([P, CCH, C], BF16, tag="wv")
            wo_sb = consts.tile([P, CCH, C], BF16, tag="wo")
            bq_sb = consts.tile([P, CCH], F32, tag="bq")
            bk_sb = consts.tile([P, CCH], F32, tag="bk")
            bv_sb = consts.tile([P, CCH], F32, tag="bv")
            bo_sb = consts.tile([P, CCH], F32, tag="bo")
            ones_sb = consts.tile([P, 1], BF16, tag="ones")
            nc.vector.memset(ones_sb[:, :], 1.0)
            id_sb = consts.tile([P, P], BF16, tag="ident")

            x_sb = acts.tile([P, CCH, NTOK], BF16, tag="x")
            c_sb = acts.tile([P, CCH, NTOK], BF16, tag="c")
            # fine-grained, dependency-ordered input DMAs: everything the
            # first k-projection group needs lands first
            wkt_r = wkt_d.rearrange("(cc p) n -> p cc n", p=P)
            wqt_r = wqt_d.rearrange("(cc p) n -> p cc n", p=P)
            wvt_r = wvt_d.rearrange("(cc p) n -> p cc n", p=P)
            wot_r = wot_d.rearrange("(cc p) n -> p cc n", p=P)
            c_r = c_d.rearrange("(cc p) n -> p cc n", p=P)
            x_r = x_d.rearrange("(cc p) n -> p cc n", p=P)
            # critical path first, in token-halves: the first k/q projection
            # groups (pair 0, tokens 0:512) unblock the first sim tiles
            nc.scalar.dma_start(out=wk_sb[:, :, 0:P], in_=wkt_r[:, :, 0:P])
            nc.sync.dma_start(out=c_sb[:, :, 0:F], in_=c_r[:, :, 0:F])
            nc.scalar.dma_start(out=wq_sb[:, :, 0:P], in_=wqt_r[:, :, 0:P])
            nc.sync.dma_start(out=x_sb[:, :, 0:F], in_=x_r[:, :, 0:F])
            nc.scalar.dma_start(out=bk_sb[:, :], in_=bk_d.rearrange("(a p) -> p a", p=P))
            nc.scalar.dma_start(out=bq_sb[:, :], in_=bq_d.rearrange("(a p) -> p a", p=P))
            nc.sync.dma_start(out=c_sb[:, :, F:NTOK], in_=c_r[:, :, F:NTOK])
            nc.sync.dma_start(out=x_sb[:, :, F:NTOK], in_=x_r[:, :, F:NTOK])
            for b_sb, b_dd in ((bv_sb, bv_d), (bo_sb, bo_d)):
                nc.scalar.dma_start(out=b_sb[:, :], in_=b_dd.rearrange("(a p) -> p a", p=P))
            # remaining weight columns (pairs 1-3) and wv/wo
            nc.scalar.dma_start(out=wk_sb[:, :, P:C], in_=wkt_r[:, :, P:C])
            nc.scalar.dma_start(out=wq_sb[:, :, P:C], in_=wqt_r[:, :, P:C])
            nc.scalar.dma_start(out=wv_sb[:, :, :], in_=wvt_r[:, :, :])
            nc.scalar.dma_start(out=wo_sb[:, :, :], in_=wot_r[:, :, :])
            nc.scalar.dma_start(out=id_sb[:, :], in_=id_d[:, :])

            # bf16 q/k staging [c-part, pair, tok]
            qst = acts.tile([P, CCH, NTOK], BF16, tag="qst")
            kst = acts.tile([P, CCH, NTOK], BF16, tag="kst")

            # v: bf16 staging [c, pair, j] then XBAR-transposed vte [j, pair, jc, c]
            vst = acts.tile([P, CCH, NTOK], BF16, tag="vst")
            vte = acts.tile([P, CCH, JCH, P], BF16, tag="vte")

            oall_sb = [
                acts.tile([P, CCH, F], BF16, tag=f"oall{ic}", name=f"oall{ic}")
                for ic in range(ICH)
            ]

            # ---- job emitters ----
            def emit_proj_group(which, pr, ih):
                """bf16 projection group -> staging (q8st/k8st fp8, vst bf16)."""
                dst, wt, bias_t, src_sb = {
                    "q": (qst, wq_sb, bq_sb, x_sb),
                    "k": (kst, wk_sb, bk_sb, c_sb),
                    "v": (vst, wv_sb, bv_sb, c_sb),
                }[which]
                ps = projP.tile([P, F], F32, tag="pj", name=f"{which}ps{pr}{ih}")
                for cc in range(CCH):
                    nc.tensor.matmul(
                        ps[:, :],
                        wt[:, cc, pr * P : (pr + 1) * P],
                        src_sb[:, cc, ih * F : (ih + 1) * F],
                        start=(cc == 0),
                        stop=(cc == CCH - 1),
                    )
                nc.vector.tensor_tensor(
                    dst[:, pr, ih * F : (ih + 1) * F],
                    ps[:, :],
                    bias_t[:, pr : pr + 1].to_broadcast([P, F]),
                    mybir.AluOpType.add,
                )

            def emit_vtr(pr, jh):
                nc.sync.dma_start_transpose(
                    vte[:, pr, jh * 4 : (jh + 1) * 4, :],
                    vst[:, pr, jh * F : (jh + 1) * F],
                )

            # split final (ic=1) o-projection: pairs 0-2 accumulate into SBUF
            # partials during the last iteration; only pair 3 remains at the tail
            partials = acts.tile([P, CCH, F], F32, tag="partials")
            opart_ps = {}

            def emit_opart(oc, ccs, finish):
                if oc not in opart_ps:
                    opart_ps[oc] = projP.tile([P, F], F32, tag="pj", name=f"opp{oc}")
                ps = opart_ps[oc]
                for cc in ccs:
                    nc.tensor.matmul(
                        ps[:, :],
                        wo_sb[:, cc, oc * P : (oc + 1) * P],
                        oall_sb[1][:, cc, :],
                        start=(cc == 0),
                        stop=(finish and cc == ccs[-1]),
                    )
                if finish:
                    nc.vector.tensor_tensor(
                        partials[:, oc, :],
                        ps[:, :],
                        bo_sb[:, oc : oc + 1].to_broadcast([P, F]),
                        mybir.AluOpType.add,
                    )

            fin_tail = acts.tile([P, CCH, F], BF16, tag="fintail")

            def emit_oproj_tail(oc):
                pool = projP if oc < 2 else attP
                ps = pool.tile([P, F], F32, tag="pj" if oc < 2 else "at", name=f"opt{oc}")
                nc.tensor.matmul(
                    ps[:, :],
                    wo_sb[:, 3, oc * P : (oc + 1) * P],
                    oall_sb[1][:, 3, :],
                    start=True,
                    stop=True,
                )
                nc.vector.tensor_tensor(
                    fin_tail[:, oc, :], ps[:, :], partials[:, oc, :], mybir.AluOpType.add
                )
                if oc in (1, 3):
                    nc.sync.dma_start(
                        out=out_d.rearrange("(cc p) n -> p cc n", p=P)[
                            :, oc - 1 : oc + 1, F : 2 * F
                        ],
                        in_=fin_tail[:, oc - 1 : oc + 1, :],
                    )

            def emit_oproj(ic, oc):
                ps = projP.tile([P, F], F32, tag="pj", name=f"ops{ic}{oc}")
                for cc in range(CCH):
                    nc.tensor.matmul(
                        ps[:, :],
                        wo_sb[:, cc, oc * P : (oc + 1) * P],
                        oall_sb[ic][:, cc, :],
                        start=(cc == 0),
                        stop=(cc == CCH - 1),
                    )
                fin = finp.tile([P, F], BF16, tag="fin", name=f"fin{ic}{oc}")
                nc.vector.tensor_tensor(
                    fin[:, :],
                    ps[:, :],
                    bo_sb[:, oc : oc + 1].to_broadcast([P, F]),
                    mybir.AluOpType.add,
                )
                nc.sync.dma_start(
                    out=out_d[oc * P : (oc + 1) * P, ic * F : (ic + 1) * F],
                    in_=fin[:, :],
                )

            def emit_sim(g, slot, pr, hb, jc, ic):
                nc.tensor.matmul(
                    g[:, slot, :],
                    kst[hb * DH : (hb + 1) * DH, pr, jc * P : (jc + 1) * P],
                    qst[hb * DH : (hb + 1) * DH, pr, ic * F : (ic + 1) * F],
                    start=True,
                    stop=True,
                )

            # ---- attn@v chain machinery (for pair `prev`, runs one iter later)
            def make_state(prev):
                pic, ppr, pes = prev
                t0 = attP.tile([P, F], F32, tag="at", name=f"at{pic}{ppr}0")
                t1 = attP.tile([P, F], F32, tag="at", name=f"at{pic}{ppr}1")
                onT = onp.tile([P, CCH, P], BF16, tag="onT", name=f"onT{pic}{ppr}")
                return {
                    "q": [(slot, jc) for jc in range(JCH) for slot in range(8)],
                    "tiles": [t0, t1],
                    "prev": prev,
                    "onT": onT,
                    # one accumulation group per bank: count matmuls per tile
                    "nmm": [0, 0],
                }

            def emit_chains_block(state, n):
                pic, ppr, pes = state["prev"]
                for _ in range(n):
                    if not state["q"]:
                        return
                    slot, jc = state["q"].pop(0)
                    half = slot // 4
                    tl = state["tiles"][half]
                    ichunk, hb = slot >> 1, slot & 1
                    col = (slot % 4 >> 1) * P + hb * DH
                    lhsT = pes[:, jc, hb, ichunk * P : (ichunk + 1) * P]
                    n = state["nmm"][half]
                    nc.tensor.matmul(
                        tl[:, col : col + DH],
                        lhsT,
                        vte[:, ppr, jc, hb * DH : (hb + 1) * DH],
                        start=(n == 0),
                        stop=False,
                        skip_group_check=True,
                    )
                    nc.tensor.matmul(
                        tl[:, 256 + (slot % 4) : 257 + (slot % 4)],
                        lhsT,
                        ones_sb[:, :],
                        start=False,
                        stop=(n + 1 == 63),
                        skip_group_check=True,
                    )
                    state["nmm"][half] = n + 2

            def emit_norm(state, half):
                pic, ppr, _ = state["prev"]
                tl = state["tiles"][half]
                onT = state["onT"]
                base = tl[:, :]
                rc = sbcp.tile([P, 4], F32, tag="rc", name=f"rc{pic}{ppr}{half}")
                nc.vector.reciprocal(
                    out=rc[:, :],
                    in_=rawap(base, [list(base.ap[0]), [1, 4]], extra_offset=256),
                )
                on0 = onT[:, half * 2, 0:DH]
                ch_ap = rawap(base, [list(base.ap[0]), [P, 2], [DH, 2], [1, DH]])
                on_ap = rawap(on0, [list(on0.ap[0]), [P, 2], [DH, 2], [1, DH]])
                rcb = rc[:, :]
                rc_ap = rawap(rcb, [list(rcb.ap[0]), [2, 2], [1, 2], [0, DH]])
                nc.vector.tensor_tensor(on_ap, ch_ap, rc_ap, mybir.AluOpType.mult)

            def emit_otr(state):
                pic, ppr, _ = state["prev"]
                nc.sync.dma_start_transpose(
                    oall_sb[pic].rearrange("p cc (a b) -> p cc a b", a=CCH)[:, ppr, :, :],
                    state["onT"].rearrange("p a b -> p (a b)"),
                )

            # ---- prologue: kq0 + kq1 around the first sim steps ----
            emit_proj_group("k", 0, 0)
            emit_proj_group("q", 0, 0)

            # global job queue with soft per-iteration targets
            jobs = []
            jobs += [lambda: emit_proj_group("k", 0, 1)]
            jobs += [lambda i=i: emit_proj_group("k", 1, i) for i in range(ICH)]
            jobs += [lambda: emit_proj_group("q", 1, 0)]
            jobs += [lambda i=i: emit_proj_group("v", 0, i) for i in range(ICH)]
            jobs += [lambda j=j: emit_vtr(0, j) for j in range(2)]
            jobs += [lambda i=i: emit_proj_group("k", 2, i) for i in range(ICH)]
            jobs += [lambda: emit_proj_group("q", 2, 0)]
            jobs += [lambda i=i: emit_proj_group("v", 1, i) for i in range(ICH)]
            jobs += [lambda j=j: emit_vtr(1, j) for j in range(2)]
            jobs += [lambda i=i: emit_proj_group("k", 3, i) for i in range(ICH)]
            jobs += [lambda: emit_proj_group("q", 3, 0)]
            jobs += [lambda i=i: emit_proj_group("v", 2, i) for i in range(ICH)]
            jobs += [lambda j=j: emit_vtr(2, j) for j in range(2)]
            jobs += [lambda: emit_proj_group("q", 0, 1)]
            jobs += [lambda: emit_proj_group("q", 1, 1)]
            jobs += [lambda i=i: emit_proj_group("v", 3, i) for i in range(ICH)]
            jobs += [lambda j=j: emit_vtr(3, j) for j in range(2)]
            jobs += [lambda: emit_proj_group("q", 2, 1)]
            jobs += [lambda: emit_proj_group("q", 3, 1)]
            jobs += [lambda o=o: emit_oproj(0, o) for o in range(4)]
            # cumulative job targets by END of iteration 0..7
            targets = [8, 15, 22, 29, 29, 32, 34, 34]

            jobs_done = 0
            iters = [(ic, pr) for ic in range(ICH) for pr in range(CCH)]
            prev = None
            state = None
            for it_idx, (ic, pr) in enumerate(iters):
                es = expp.tile([P, JCH, 2, F], BF16, tag="es", name=f"es{ic}{pr}")
                state = make_state(prev) if prev is not None else None
                tgt = targets[it_idx]
                start_jobs = jobs_done
                for t in range(JCH):  # 8 sim tiles per iteration
                    g = simP.tile([P, 2, F], F32, tag="sim", name=f"g{ic}{pr}{t}")
                    for hb in range(2):
                        emit_sim(g, hb, pr, hb, t, ic)
                    nc.scalar.activation(
                        out=es[:, t, :, :],
                        in_=g[:, :, :],
                        func=mybir.ActivationFunctionType.Exp,
                        scale=SCALE,
                    )
                    # spread this iteration's job quota evenly across the 8 steps
                    want = start_jobs + (tgt - start_jobs) * (t + 1) // JCH
                    while jobs_done < want and jobs_done < len(jobs):
                        jobs[jobs_done]()
                        jobs_done += 1
                    if it_idx == len(iters) - 1:
                        # last iteration: drain prev fast, then self-chase
                        # this pair's chains one exp-step behind
                        if t <= 3:
                            emit_chains_block(state, 16)
                        elif t == 4:
                            emit_chains_block(state, 99)
                            emit_norm(state, 0)
                            emit_norm(state, 1)
                            emit_otr(state)
                            self_state = make_state((ic, pr, es))
                            emit_chains_block(self_state, 32)  # backlog jc 0..3
                        else:
                            emit_chains_block(self_state, 8)  # jc = t-1
                            if t == 5:
                                emit_opart(0, [0, 1], False)
                            elif t == 6:
                                emit_opart(1, [0, 1], False)
                            elif t == 7:
                                emit_opart(0, [2], True)
                                emit_opart(1, [2], True)
                    elif state is not None and t >= 2:
                        emit_chains_block(state, 16)
                        if t == 5:
                            emit_norm(state, 0)
                            emit_norm(state, 1)
                            emit_otr(state)
                prev = (ic, pr, es)

            # ---- tail: last jc of the self-chase + final o-projection ----
            emit_chains_block(self_state, 99)
            wps2 = projP.tile([P, F], F32, tag="pj", name="warmps2")
            for i in range(24):
                nc.tensor.matmul(
                    wps2[0:DH, 0:P],
                    warm_sb[:, :],
                    wq_sb[:, 0, 0:P],
                    start=True,
                    stop=True,
                )
            emit_norm(self_state, 0)
            emit_norm(self_state, 1)
            # PE-transpose (no DMA latency) for the final pair: onT -> PSUM,
            # then one DVE copy into oall
            pic, ppr, _ = self_state["prev"]
            trp = attP.tile([P, CCH, P], BF16, tag="at", name="trtail")
            for k in range(CCH):
                nc.tensor.matmul(
                    trp[:, k, :],
                    self_state["onT"][:, k, :],
                    id_sb[:, :],
                    is_transpose=True,
                    start=(k == 0),
                    stop=(k == CCH - 1),
                    skip_group_check=True,
                )
            nc.vector.tensor_copy(
                oall_sb[pic][:, ppr, :], trp.rearrange("p a b -> p (a b)")
            )
            emit_opart(2, [0, 1, 2], True)
            emit_opart(3, [0, 1, 2], True)
            # p-state filler: bridge the DVE copy window so the final
            # o-projection matmuls run at full clock
            for i in range(16):
                nc.tensor.matmul(
                    wps2[0:DH, 0:P],
                    warm_sb[:, :],
                    wq_sb[:, 0, 0:P],
                    start=True,
                    stop=True,
                )
            for oc in range(4):
                emit_oproj_tail(oc)

    nc.compile()
    return nc


def prep_inputs(x, context, Wq, bq, Wk, bk, Wv, bv, Wo, bo):
    xb = np.asarray(x, np.float32).reshape(B, C, NTOK).astype(NPBF16)
    cb = np.asarray(context, np.float32).reshape(B, C, NTOK).astype(NPBF16)
    wqt = np.ascontiguousarray(np.asarray(Wq, np.float32).T).astype(NPBF16)
    wkt = np.ascontiguousarray(np.asarray(Wk, np.float32).T).astype(NPBF16)
    wvt = np.ascontiguousarray(np.asarray(Wv, np.float32).T).astype(NPBF16)
    wot = np.ascontiguousarray(np.asarray(Wo, np.float32).T).astype(NPBF16)
    in_maps = []
    for b in range(B):
        in_maps.append(
            {
                "x": np.ascontiguousarray(xb[b]),
                "ctx": np.ascontiguousarray(cb[b]),
                "wqt": wqt,
                "wkt": wkt,
                "wvt": wvt,
                "wot": wot,
                "bq": np.asarray(bq, np.float32),
                "bk": np.asarray(bk, np.float32),
                "bv": np.asarray(bv, np.float32),
                "bo": np.asarray(bo, np.float32),
                "ident": np.eye(P, dtype=NPBF16),
            }
        )
    return in_maps


_NC = None


def _get_nc():
    global _NC
    if _NC is None:
        _NC = build_nc()
    return _NC


def kernel(x, context, Wq, bq, Wk, bk, Wv, bv, Wo, bo):
    from concourse.bass_utils import run_bass_kernel_spmd

    nc = _get_nc()
    in_maps = prep_inputs(x, context, Wq, bq, Wk, bk, Wv, bv, Wo, bo)
    br = run_bass_kernel_spmd(nc, in_maps, list(range(B)))
    out = np.stack([np.asarray(br.results[b]["out"], np.float32) for b in range(B)])
    return out.reshape(B, C, 32, 32)
